# revision 1
# baseline (speedup 1.0000x reference)
"""Inverse separable wavelet synthesis (stride-2 transposed conv, 9 taps,
36 -> 12 -> 4 channels, 256x256 -> 512x512) on 8 trn2 NeuronCores.

Formulation: both passes are dense matmuls against host-precomputed banded
operator matrices A_beta [256 in, 512 out] (one per wavelet band), with
symmetric padding + border-mask sign folded into the operators.  H == W so
both passes share the same operators.  The Y pass (contract over h) runs
FIRST because its moving operand is the input in its natural DRAM layout
[h partitions, (w, c) free] - fully contiguous DMA (the X-pass-first variant
needs w on partitions, which forces 144-byte-granule descriptor-bound DMA).

    u[b,n,w,q]   = sum_{by,i} A_by[i,n] * x[b,i,w,9*g2+3*by+bx],  q=(g2,bx)
    out[b,n,m,g2] = sum_{bx,j} A_bx[j,m] * u[b,n,j,3*g2+bx]

Per-core pipeline (pure batch parallelism, 2 images per core):
  stage Y : lhsT = operator window [68 h-in, 128 h2-out] (four overlapping
            input windows, one per output block -> single k-tile each),
            moving = x [h-in part, (g2, w) free] -> PSUM [h2, (g2, w)]
  PE transpose 128x128: u [h2, (q, w)] -> u' [w, (q, h2)]
  stage X : lhsT = operator block [128 w-in, 128 w2-out], moving = u'
            [w-in part, (g2, h2) free] -> PSUM [w2, (g2, h2)]
  PE transpose 128x128: v [w2, (c, h2)] -> osb [h2, (w2, c)] -> DRAM rows
All matmul operands are float32r (fp32 with 11-bit mantissa, full PE rate);
accumulation is exact fp32 in PSUM.  Operator coefficients are dyadic
rationals - exact in f32r; only the input and intermediates get rounded
(~3e-4 relative output error).
"""

import numpy as np
from contextlib import ExitStack

import concourse.bass as bass
import concourse.bacc as bacc
import concourse.mybir as mybir
import concourse.tile as tile
from concourse.bass_utils import run_bass_kernel_spmd

B, H, W, C = 16, 256, 256, 36
NCORES = 8
BPC = B // NCORES  # batches per core
W2 = 2 * W
H2 = 2 * H
F32 = mybir.dt.float32
F32R = mybir.dt.float32r

SMOOTH = [0.0, 0.0, 1.0 / 16.0, 0.5, 14.0 / 16.0, 0.5, 1.0 / 16.0, 0.0, 0.0]
EVEN = [-1.0 / 128.0, -1.0 / 16.0, -10.0 / 64.0, -7.0 / 16.0, 85.0 / 64.0,
        -7.0 / 16.0, -10.0 / 64.0, -1.0 / 16.0, -1.0 / 128.0]
ODD = [1.0 / 256.0, 1.0 / 32.0, 15.0 / 128.0, 17.0 / 32.0, 0.0,
       -17.0 / 32.0, -15.0 / 128.0, -1.0 / 32.0, -1.0 / 256.0]

# Stage X: which 128-row k-tiles of u' feed each 128-col w2 output block
# (out block n covers in rows [64n-2, 64n+65]).
KTS = {0: (0,), 1: (0, 1), 2: (0, 1), 3: (1,)}
# Stage Y input windows (one 68-row window per 128-row h2 output block).
W0 = [0, 62, 126, 188]
KW = 68


def _build_operator_full():
    """[3 bands, 256 in-rows, 512 out-cols] float64 folded operator."""
    inv = np.array([SMOOTH, EVEN, ODD], dtype=np.float64)
    S = 256
    Sp = S + 6
    j = np.arange(Sp)[:, None]
    m = np.arange(2 * S)[None, :]
    t = m + 10 - 2 * j
    valid = (t >= 0) & (t <= 8)
    P = np.zeros((3, Sp, 2 * S))
    for b in range(3):
        P[b][valid] = inv[b][t[valid]]
    # border mask: odd band negated on the 3-wide padded border
    P[2, [0, 1, 2, Sp - 3, Sp - 2, Sp - 1], :] *= -1.0
    # fold symmetric padding: pad[0..2]=x[2],x[1],x[0]; pad[-3:]=x[-1],x[-2],x[-3]
    A = P[:, 3:3 + S].copy()
    A[:, 2] += P[:, 0]
    A[:, 1] += P[:, 1]
    A[:, 0] += P[:, 2]
    A[:, S - 1] += P[:, Sp - 3]
    A[:, S - 2] += P[:, Sp - 2]
    A[:, S - 3] += P[:, Sp - 1]
    return A


def _build_operator_array():
    """Stage-X operator: [3 bands, 2 ktiles, 128 in-rows, 512 out-cols] f32."""
    A = _build_operator_full()
    return np.ascontiguousarray(A.reshape(3, 2, 128, 512).astype(np.float32))


def _build_operator_windows():
    """Stage-Y operator: [3 bands, 4 blocks, 68 in-rows, 128 out-cols] f32."""
    A = _build_operator_full()
    out = np.zeros((3, 4, KW, 128), np.float64)
    for blk in range(4):
        out[:, blk] = A[:, W0[blk]:W0[blk] + KW, blk * 128:(blk + 1) * 128]
    return np.ascontiguousarray(out.astype(np.float32))


def _build_program(repeat=1):
    nc = bacc.Bacc("TRN2", target_bir_lowering=False)
    x = nc.declare_dram_parameter("x", [BPC, H, W, C], F32R, isOutput=False)
    a_w = nc.declare_dram_parameter("a_w", [3, 4, KW, 128], F32R, isOutput=False)
    a_op = nc.declare_dram_parameter("a_op", [3, 2, 128, W2], F32R, isOutput=False)
    ident = nc.declare_dram_parameter("ident", [128, 128], F32R, isOutput=False)
    out = nc.declare_dram_parameter("out", [BPC, H2, W2, 4], F32, isOutput=True)

    with tile.TileContext(nc) as tc, ExitStack() as ctx:
        const = ctx.enter_context(tc.tile_pool(name="const", bufs=1))
        xpool = ctx.enter_context(tc.tile_pool(name="xp", bufs=4))
        upool = ctx.enter_context(tc.tile_pool(name="up", bufs=2))
        vpool = ctx.enter_context(tc.tile_pool(name="vp", bufs=1))
        wpool = ctx.enter_context(tc.tile_pool(name="wp", bufs=3))
        opool = ctx.enter_context(tc.tile_pool(name="op", bufs=2))
        psY = ctx.enter_context(tc.tile_pool(name="psY", bufs=3, space="PSUM"))
        psT = ctx.enter_context(tc.tile_pool(name="psT", bufs=2, space="PSUM"))
        psX = ctx.enter_context(tc.tile_pool(name="psX", bufs=2, space="PSUM"))

        aw_sb = {}
        for beta in range(3):
            for blk in range(4):
                t = const.tile([KW, 128], F32R, name=f"aw_{beta}_{blk}",
                               tag=f"aw_{beta}_{blk}")
                nc.sync.dma_start(t[:], a_w[beta, blk])
                aw_sb[beta, blk] = t
        a_sb = {}
        for beta in range(3):
            for kt in range(2):
                t = const.tile([128, W2], F32R, name=f"a_{beta}_{kt}",
                               tag=f"a_{beta}_{kt}")
                nc.sync.dma_start(t[:], a_op[beta, kt])
                a_sb[beta, kt] = t
        ident_sb = const.tile([128, 128], F32R, name="ident_sb", tag="ident")
        nc.sync.dma_start(ident_sb[:], ident[:])

        for rep in range(repeat):
          for b in range(BPC):
            rb = rep * BPC + b
            # u[h2blk]: [128 h2, (q=12 ch, w=256)], q = 3*g2 + bx
            u = {}
            for blk in range(4):
                u[blk] = upool.tile([128, 12 * W], F32R,
                                    name=f"u_{rb}_{blk}", tag=f"u_{blk % 2}")
            # ---- stage Y: contract h (natural-layout loads) ----
            for wc in range(2):
                xw = {}
                for win in range(4):
                    xt = xpool.tile([KW, 128 * C], F32R,
                                    name=f"x_{rb}_{wc}_{win}", tag="x")
                    src = x[b, W0[win]:W0[win] + KW,
                            wc * 128:(wc + 1) * 128, :]
                    nc.sync.dma_start(
                        xt.rearrange("h (w c) -> h w c", c=C), src)
                    xw[win] = xt
                for blk in range(4):
                    uv = u[blk].rearrange("p (q w) -> p q w", q=12)
                    for bx in range(3):
                        ps = psY.tile([128, 512], F32,
                                      name=f"psY_{rb}_{wc}_{blk}_{bx}",
                                      tag="psY")
                        psv = ps.rearrange("p (g w) -> p g w", g=4)
                        for i, by in enumerate(range(3)):
                            # channels c = 9*g2 + 3*by + bx
                            rhs = xw[blk].rearrange(
                                "h (w g2 e c) -> h g2 e c w",
                                g2=4, e=3, c=3)[:, :, by, bx, :]
                            nc.tensor.matmul(psv, aw_sb[by, blk][:], rhs,
                                             start=(i == 0), stop=(i == 2))
                        # scatter into u: q = 3*g2 + bx
                        dst = u[blk].rearrange(
                            "p (g2 e w) -> p e g2 w", g2=4, e=3)[
                                :, bx, :, wc * 128:(wc + 1) * 128]
                        nc.vector.tensor_copy(out=dst, in_=psv)
            # ---- mid transposes + stage X, streamed per h2 block ----
            # v[w2blk]: [128 w2, (c=4, h2=512)]
            v = {}
            for blk in range(4):
                v[blk] = vpool.tile([128, 4 * H2], F32R,
                                    name=f"v_{rb}_{blk}", tag=f"v_{blk}")
            for h2b in range(4):
                # transpose u[h2b] [h2, (q, w)] -> up[wt] [w, (q, h2-slice)]
                up = {}
                for wt in range(2):
                    up[wt] = wpool.tile([128, 12 * 128], F32R,
                                        name=f"up_{rb}_{h2b}_{wt}", tag="up")
                uvb = u[h2b].rearrange("p (q w) -> p q w", q=12)
                for wt in range(2):
                    for q4 in range(3):
                        pt = psT.tile([128, 512], F32R,
                                      name=f"psT_{rb}_{h2b}_{wt}_{q4}",
                                      tag="psT")
                        for i in range(4):
                            q = q4 * 4 + i
                            nc.tensor.transpose(
                                pt[:, i * 128:(i + 1) * 128],
                                uvb[:, q, wt * 128:(wt + 1) * 128],
                                ident_sb[:])
                        dst = up[wt].rearrange("p (q h) -> p q h", q=12)[
                            :, q4 * 4:(q4 + 1) * 4, :]
                        src = pt.rearrange("p (q h) -> p q h", q=4)
                        if (wt + q4) % 2 == 0:
                            nc.vector.tensor_copy(out=dst, in_=src)
                        else:
                            nc.scalar.copy(out=dst, in_=src)
                # stage X for this h2 slice
                for w2b in range(4):
                    ps = psX.tile([128, 512], F32,
                                  name=f"psX_{rb}_{h2b}_{w2b}", tag="psX")
                    psv = ps.rearrange("p (g h) -> p g h", g=4)
                    mms = [(bx, kt) for bx in range(3) for kt in KTS[w2b]]
                    for i, (bx, kt) in enumerate(mms):
                        lhsT = a_sb[bx, kt][:, w2b * 128:(w2b + 1) * 128]
                        # q = 3*g2 + bx -> fixed bx, g2 strided by 3
                        rhs = up[kt].rearrange(
                            "p (g2 e h) -> p e g2 h", g2=4, e=3)[:, bx, :, :]
                        nc.tensor.matmul(psv, lhsT, rhs,
                                         start=(i == 0),
                                         stop=(i == len(mms) - 1))
                    dst = v[w2b].rearrange("p (c h) -> p c h", c=4)[
                        :, :, h2b * 128:(h2b + 1) * 128]
                    if (h2b + w2b) % 2 == 0:
                        nc.scalar.copy(out=dst, in_=psv)
                    else:
                        nc.vector.tensor_copy(out=dst, in_=psv)
            # ---- output transposes: v [w2, (c, h2)] -> osb [h2, (w2, c)] ----
            for h2t in range(4):
                osb = opool.tile([128, W2 * 4], F32, name=f"osb_{rb}_{h2t}",
                                 tag="osb")
                osbv = osb.rearrange("p (w c) -> p c w", c=4)
                for w2b in range(4):
                    pt = psT.tile([128, 512], F32,
                                  name=f"psO_{rb}_{h2t}_{w2b}", tag="psT")
                    vv = v[w2b].rearrange("p (c h) -> p c h", c=4)
                    for c in range(4):
                        nc.tensor.transpose(
                            pt[:, c * 128:(c + 1) * 128].bitcast(F32R),
                            vv[:, c, h2t * 128:(h2t + 1) * 128],
                            ident_sb[:])
                    dst = osbv[:, :, w2b * 128:(w2b + 1) * 128]
                    src = pt.rearrange("p (c w) -> p c w", c=4)
                    if w2b % 2 == 0:
                        nc.vector.tensor_copy(out=dst, in_=src)
                    else:
                        nc.scalar.copy(out=dst, in_=src)
                dstd = out[b, h2t * 128:(h2t + 1) * 128, :, :].rearrange(
                    "h w c -> h (w c)")
                nc.sync.dma_start(dstd, osb[:])
    nc.compile()
    return nc


def _round_fp32r(x):
    """Round fp32 array to fp32r (fp32 with 11-bit mantissa, RNE) on host."""
    b = x.view(np.uint32).astype(np.uint64)
    b = (b + 0x7FF + ((b >> 12) & 1)) & ~np.uint64(0xFFF)
    return b.astype(np.uint32).view(np.float32)


_PROGRAMS = {}


def _get_program(repeat=1, mode=None, phases=None):
    if repeat not in _PROGRAMS:
        _PROGRAMS[repeat] = _build_program(repeat)
    return _PROGRAMS[repeat]


def _host_inputs(inputs):
    a4 = _build_operator_array()
    aw = _build_operator_windows()
    identity = np.ascontiguousarray(np.eye(128, dtype=np.float32))
    shards = _round_fp32r(inputs).reshape(NCORES, BPC, H, W, C)
    return [{"x": np.ascontiguousarray(shards[c]), "a_op": a4, "a_w": aw,
             "ident": identity} for c in range(NCORES)]


def _run(inputs, trace=False, tmpdir=None, repeat=1, mode=None):
    """Returns (full output [16,512,512,4], BassKernelResults)."""
    inputs = np.ascontiguousarray(np.asarray(inputs, dtype=np.float32))
    assert inputs.shape == (B, H, W, C), inputs.shape
    nc = _get_program(repeat)
    in_maps = _host_inputs(inputs)
    res = run_bass_kernel_spmd(nc, in_maps, core_ids=list(range(NCORES)),
                               trace=trace, tmpdir=tmpdir)
    outs = [np.asarray(res.results[c]["out"]) for c in range(NCORES)]
    full = np.concatenate(outs, axis=0).astype(np.float32)
    return full, res


def kernel(inputs):
    full, _ = _run(inputs)
    return full



# revision 6
# speedup vs baseline: 3.5249x; 3.5249x over previous
"""Inverse separable wavelet synthesis (stride-2 transposed conv, 9 taps,
36 -> 12 -> 4 channels, 256x256 -> 512x512) on 8 trn2 NeuronCores.

Formulation: both passes are matmuls against the SAME host-precomputed
banded operator matrix A_band [256 in, 512 out] (symmetric padding +
border-mask sign folded in; H == W so both passes share the operators).

Data-stationary layout (the key trick): the *image data* is the
stationary (lhsT) operand and the *operator* is the moving (rhs)
operand.  The PE computes out[m,n] = lhsT.T @ rhs, so the psum
partition dim m comes from the data tile's free axis:

  stage Y: lhsT = x[h-rows, w-cols] (per channel), rhs = A_by[h, h2]
           -> psum [w, h2]   : u arrives ALREADY transposed for stage X
  stage X: lhsT = u[w-rows, h2-cols], rhs = A_bx[w, w2]
           -> psum [h2, w2]  : final output orientation, no transposes

Banded structure is exploited on the free axis: the k-split over two
128-row h (or w) tiles only needs output columns [0,260) / [252,512)
(5-tap window), so accumulation passes stream ~260 columns, not 512.
The first matmul of each group streams the full 512 columns so every
psum element is initialized by a start=True write (rhs is zero outside
the band, so this is exact).

All matmul operands are fp16: the operator coefficients are dyadic
rationals (exact in fp16); only x and the intermediate u are rounded
(~1e-3 relative output error).  Accumulation is exact fp32 in PSUM.
Input is converted to fp16 on the host, halving input DMA bytes; all
DMA tiles have 128 partitions so transfers spray across all 16 DMA
engines (a 68-partition tile would use only 4 = gcd-style spray rule).
"""

import numpy as np
from contextlib import ExitStack

import concourse.bass as bass
import concourse.bacc as bacc
import concourse.mybir as mybir
import concourse.tile as tile
from concourse.bass_utils import run_bass_kernel_spmd

B, H, W, C = 16, 256, 256, 36
NCORES = 8
BPC = B // NCORES  # images per core
W2 = 2 * W
H2 = 2 * H
F32 = mybir.dt.float32
FP16 = mybir.dt.float16

SMOOTH = [0.0, 0.0, 1.0 / 16.0, 0.5, 14.0 / 16.0, 0.5, 1.0 / 16.0, 0.0, 0.0]
EVEN = [-1.0 / 128.0, -1.0 / 16.0, -10.0 / 64.0, -7.0 / 16.0, 85.0 / 64.0,
        -7.0 / 16.0, -10.0 / 64.0, -1.0 / 16.0, -1.0 / 128.0]
ODD = [1.0 / 256.0, 1.0 / 32.0, 15.0 / 128.0, 17.0 / 32.0, 0.0,
       -17.0 / 32.0, -15.0 / 128.0, -1.0 / 32.0, -1.0 / 256.0]

# Output-column ranges fed by each 128-row k-tile: A[i, m] nonzero iff
# m in [2i-4, 2i+4], so k-tile 0 (rows 0..127) covers m < 259 and
# k-tile 1 (rows 128..255) covers m >= 252.
KT_COLS = {0: (0, 260), 1: (252, 512)}


def _build_operator_full():
    """[3 bands, 256 in-rows, 512 out-cols] float64 folded operator."""
    inv = np.array([SMOOTH, EVEN, ODD], dtype=np.float64)
    S = 256
    Sp = S + 6
    j = np.arange(Sp)[:, None]
    m = np.arange(2 * S)[None, :]
    t = m + 10 - 2 * j
    valid = (t >= 0) & (t <= 8)
    P = np.zeros((3, Sp, 2 * S))
    for b in range(3):
        P[b][valid] = inv[b][t[valid]]
    # border mask: odd band negated on the 3-wide padded border
    P[2, [0, 1, 2, Sp - 3, Sp - 2, Sp - 1], :] *= -1.0
    # fold symmetric padding: pad[0..2]=x[2],x[1],x[0]; pad[-3:]=x[-1],x[-2],x[-3]
    A = P[:, 3:3 + S].copy()
    A[:, 2] += P[:, 0]
    A[:, 1] += P[:, 1]
    A[:, 0] += P[:, 2]
    A[:, S - 1] += P[:, Sp - 3]
    A[:, S - 2] += P[:, Sp - 2]
    A[:, S - 3] += P[:, Sp - 1]
    return A


def _build_operator_tiles():
    """[3 bands, 2 ktiles, 128 in-rows, 512 out-cols] fp16."""
    A = _build_operator_full()
    At = A.reshape(3, 2, 128, 512)
    return np.ascontiguousarray(At.astype(np.float16))


def _build_program(repeat=1):
    nc = bacc.Bacc("TRN2", target_bir_lowering=False)
    x = nc.declare_dram_parameter("x", [BPC, H, W, C], FP16, isOutput=False)
    a_t = nc.declare_dram_parameter("a_t", [3, 2, 128, 512], FP16,
                                    isOutput=False)
    out = nc.declare_dram_parameter("out", [BPC, H2, W2, 4], F32,
                                    isOutput=True)

    # accumulation order for one psum tile: (band, ktile)
    MM_ORDER = [(0, 0), (1, 0), (2, 0), (0, 1), (1, 1), (2, 1)]

    with tile.TileContext(nc) as tc, ExitStack() as ctx:
        const = ctx.enter_context(tc.tile_pool(name="const", bufs=1))
        xpool = ctx.enter_context(tc.tile_pool(name="xp", bufs=2))
        upool = ctx.enter_context(tc.tile_pool(name="up", bufs=2))
        opool = ctx.enter_context(tc.tile_pool(name="op", bufs=1))
        psY = ctx.enter_context(tc.tile_pool(name="psY", bufs=4,
                                             space="PSUM"))
        psX = ctx.enter_context(tc.tile_pool(name="psX", bufs=4,
                                             space="PSUM"))

        # operator tiles [128, 512] fp16, shared by both stages
        at_sb = {}
        for band in range(3):
            for kt in range(2):
                t = const.tile([128, 512], FP16, name=f"at_{band}_{kt}",
                               tag=f"at_{band}_{kt}")
                nc.sync.dma_start(t[:], a_t[band, kt])
                at_sb[band, kt] = t

        def _vcopy(out, in_):
            nc.vector.tensor_copy(out=out, in_=in_)

        def _scopy(out, in_):
            nc.scalar.copy(out=out, in_=in_)

        # gpsimd (Pool) cannot read PSUM, so only DVE + Act move psum data
        copy_fns = [_vcopy, _scopy]
        eng_i = 0

        def next_engine():
            nonlocal eng_i
            e = copy_fns[eng_i % 2]
            eng_i += 1
            return e

        def load_image(rb, b):
            """x[b] -> two SBUF tiles [128 h, (w c)] fp16, DMA'd in
            wc-halves so stage Y can start after the first half."""
            ts = {}
            for ht in range(2):
                ts[ht] = xpool.tile([128, 256 * C], FP16,
                                    name=f"x_{rb}_{ht}", tag=f"x_{ht}")
            for wc in range(2):
                for ht in range(2):
                    dst = ts[ht].rearrange("h (w c) -> h w c", c=C)[
                        :, wc * 128:(wc + 1) * 128, :]
                    src = x[b, ht * 128:(ht + 1) * 128,
                            wc * 128:(wc + 1) * 128, :]
                    nc.sync.dma_start(dst, src)
            return ts

        xt = {}
        for rep in range(repeat):
          for b in range(BPC):
            rb = rep * BPC + b
            if b == 0 and rep == 0:
                xt[rb] = load_image(rb, b)
            if rb + 1 < repeat * BPC:
                xt[rb + 1] = load_image(rb + 1, (rb + 1) % BPC)

            xv = {ht: xt[rb][ht].rearrange("h (w c) -> h w c", c=C)
                  for ht in range(2)}

            # ---- stage Y: contract h -> u[q][wc] = [128 w, 512 h2] ----
            u_sb = {}
            for wc in range(2):
                for g2 in range(4):
                    for bx in range(3):
                        q = 3 * g2 + bx
                        P = psY.tile([128, 512], F32,
                                     name=f"psY_{rb}_{wc}_{q}", tag="psY")
                        for i, (by, ht) in enumerate(MM_ORDER):
                            c0q, c1q = (0, 512) if i == 0 else KT_COLS[ht]
                            c0 = 9 * g2 + 3 * by + bx
                            lhsT = xv[ht][:, wc * 128:(wc + 1) * 128, c0]
                            nc.tensor.matmul(
                                P[:, c0q:c1q], lhsT,
                                at_sb[by, ht][:, c0q:c1q],
                                start=(i == 0), stop=(i == 5))
                        ut = upool.tile([128, 512], FP16,
                                        name=f"u_{rb}_{q}_{wc}",
                                        tag=f"u_{q}_{wc}")
                        next_engine()(ut[:], P[:])
                        u_sb[q, wc] = ut

            # ---- stage X: contract w -> psum [128 h2, 512 w2] ----
            for hb in range(4):
                osb = opool.tile([128, W2 * 4], F32, name=f"osb_{rb}_{hb}",
                                 tag=f"osb_{hb}")
                osbv = osb.rearrange("p (w g) -> p g w", g=4)
                for g2 in range(4):
                    V = psX.tile([128, 512], F32,
                                 name=f"psX_{rb}_{hb}_{g2}", tag="psX")
                    for i, (bx, wc) in enumerate(MM_ORDER):
                        c0q, c1q = (0, 512) if i == 0 else KT_COLS[wc]
                        lhsT = u_sb[3 * g2 + bx, wc][
                            :, hb * 128:(hb + 1) * 128]
                        nc.tensor.matmul(
                            V[:, c0q:c1q], lhsT,
                            at_sb[bx, wc][:, c0q:c1q],
                            start=(i == 0), stop=(i == 5))
                    next_engine()(osbv[:, g2, :], V[:])
                dstd = out[b, hb * 128:(hb + 1) * 128, :, :].rearrange(
                    "h w c -> h (w c)")
                nc.sync.dma_start(dstd, osb[:])
    nc.compile()
    return nc


_PROGRAMS = {}


def _get_program(repeat=1):
    if repeat not in _PROGRAMS:
        _PROGRAMS[repeat] = _build_program(repeat)
    return _PROGRAMS[repeat]


def _host_inputs(inputs):
    at = _build_operator_tiles()
    shards = inputs.astype(np.float16).reshape(
        NCORES, BPC, H, W, C)
    return [{"x": np.ascontiguousarray(shards[c]), "a_t": at}
            for c in range(NCORES)]


def _run(inputs, trace=False, tmpdir=None, repeat=1):
    """Returns (full output [16,512,512,4], BassKernelResults)."""
    inputs = np.ascontiguousarray(np.asarray(inputs, dtype=np.float32))
    assert inputs.shape == (B, H, W, C), inputs.shape
    nc = _get_program(repeat)
    in_maps = _host_inputs(inputs)
    res = run_bass_kernel_spmd(nc, in_maps, core_ids=list(range(NCORES)),
                               trace=trace, tmpdir=tmpdir)
    outs = [np.asarray(res.results[c]["out"]) for c in range(NCORES)]
    full = np.concatenate(outs, axis=0).astype(np.float32)
    return full, res


def kernel(inputs):
    full, _ = _run(inputs)
    return full


# revision 12
# speedup vs baseline: 3.9256x; 1.1137x over previous
"""Inverse separable wavelet synthesis (stride-2 transposed conv, 9 taps,
36 -> 12 -> 4 channels, 256x256 -> 512x512) on 8 trn2 NeuronCores.

Formulation: both passes are matmuls against the SAME host-precomputed
banded operator matrix A_band [256 in, 512 out] (symmetric padding +
border-mask sign folded in; H == W so both passes share the operators).

Data-stationary layout (the key trick): the *image data* is the
stationary (lhsT) operand and the *operator* is the moving (rhs)
operand.  The PE computes out[m,n] = lhsT.T @ rhs, so the psum
partition dim m comes from the data tile's free axis:

  stage Y: lhsT = x[h-rows, w-cols] (per channel), rhs = A_by[h, h2]
           -> psum [w, h2]   : u arrives ALREADY transposed for stage X
  stage X: lhsT = u[w-rows, h2-cols], rhs = A_bx[w, w2]
           -> psum [h2, w2]  : final output orientation, no transposes

Banded structure is exploited on the free axis: the k-split over two
128-row h (or w) tiles only needs output columns [0,260) / [252,512)
(5-tap window), so accumulation passes stream ~260 columns, not 512.
The first matmul of each group streams the full 512 columns so every
psum element is initialized by a start=True write (rhs is zero outside
the band, so this is exact).

All matmul operands are fp16: the operator coefficients are dyadic
rationals (exact in fp16); only x and the intermediate u are rounded
(~1e-3 relative output error).  Accumulation is exact fp32 in PSUM.
Input is converted to fp16 on the host, halving input DMA bytes; all
DMA tiles have 128 partitions so transfers spray across all 16 DMA
engines (a 68-partition tile would use only 4 = gcd-style spray rule).
"""

import numpy as np
from contextlib import ExitStack

import concourse.bass as bass
import concourse.bacc as bacc
import concourse.mybir as mybir
import concourse.tile as tile
from concourse.bass_utils import run_bass_kernel_spmd

B, H, W, C = 16, 256, 256, 36
NCORES = 8
BPC = B // NCORES  # images per core
W2 = 2 * W
H2 = 2 * H
F32 = mybir.dt.float32
FP16 = mybir.dt.float16

SMOOTH = [0.0, 0.0, 1.0 / 16.0, 0.5, 14.0 / 16.0, 0.5, 1.0 / 16.0, 0.0, 0.0]
EVEN = [-1.0 / 128.0, -1.0 / 16.0, -10.0 / 64.0, -7.0 / 16.0, 85.0 / 64.0,
        -7.0 / 16.0, -10.0 / 64.0, -1.0 / 16.0, -1.0 / 128.0]
ODD = [1.0 / 256.0, 1.0 / 32.0, 15.0 / 128.0, 17.0 / 32.0, 0.0,
       -17.0 / 32.0, -15.0 / 128.0, -1.0 / 32.0, -1.0 / 256.0]

# Output-column ranges fed by each 128-row k-tile: A[i, m] nonzero iff
# m in [2i-4, 2i+4], so k-tile 0 (rows 0..127) covers m < 259 and
# k-tile 1 (rows 128..255) covers m >= 252.
KT_COLS = {0: (0, 260), 1: (252, 512)}


def _build_operator_full():
    """[3 bands, 256 in-rows, 512 out-cols] float64 folded operator."""
    inv = np.array([SMOOTH, EVEN, ODD], dtype=np.float64)
    S = 256
    Sp = S + 6
    j = np.arange(Sp)[:, None]
    m = np.arange(2 * S)[None, :]
    t = m + 10 - 2 * j
    valid = (t >= 0) & (t <= 8)
    P = np.zeros((3, Sp, 2 * S))
    for b in range(3):
        P[b][valid] = inv[b][t[valid]]
    # border mask: odd band negated on the 3-wide padded border
    P[2, [0, 1, 2, Sp - 3, Sp - 2, Sp - 1], :] *= -1.0
    # fold symmetric padding: pad[0..2]=x[2],x[1],x[0]; pad[-3:]=x[-1],x[-2],x[-3]
    A = P[:, 3:3 + S].copy()
    A[:, 2] += P[:, 0]
    A[:, 1] += P[:, 1]
    A[:, 0] += P[:, 2]
    A[:, S - 1] += P[:, Sp - 3]
    A[:, S - 2] += P[:, Sp - 2]
    A[:, S - 3] += P[:, Sp - 1]
    return A


def _build_operator_tiles():
    """[3 bands, 2 ktiles, 128 in-rows, 512 out-cols] fp16."""
    A = _build_operator_full()
    At = A.reshape(3, 2, 128, 512)
    return np.ascontiguousarray(At.astype(np.float16))


def _build_program(repeat=1):
    nc = bacc.Bacc("TRN2", target_bir_lowering=False)
    x = nc.declare_dram_parameter("x", [BPC, H, W, C], FP16, isOutput=False)
    a_t = nc.declare_dram_parameter("a_t", [3, 2, 128, 512], FP16,
                                    isOutput=False)
    out = nc.declare_dram_parameter("out", [BPC, H2, W2, 4], FP16,
                                    isOutput=True)

    # accumulation order for one psum tile: (band, ktile)
    MM_ORDER = [(0, 0), (1, 0), (2, 0), (0, 1), (1, 1), (2, 1)]

    with tile.TileContext(nc) as tc, ExitStack() as ctx:
        const = ctx.enter_context(tc.tile_pool(name="const", bufs=1))
        xpool = ctx.enter_context(tc.tile_pool(name="xp", bufs=2))
        upool = ctx.enter_context(tc.tile_pool(name="up", bufs=2))
        opool = ctx.enter_context(tc.tile_pool(name="op", bufs=1))
        psY = ctx.enter_context(tc.tile_pool(name="psY", bufs=4,
                                             space="PSUM"))
        psX = ctx.enter_context(tc.tile_pool(name="psX", bufs=4,
                                             space="PSUM"))

        # operator tiles [128, 512] fp16, shared by both stages.  kt=0
        # tiles load first: the first psum group's ht=0 matmuls need only
        # those plus the first x chunk.
        at_sb = {}
        for kt in range(2):
            for band in range(3):
                t = const.tile([128, 512], FP16, name=f"at_{band}_{kt}",
                               tag=f"at_{band}_{kt}")
                at_sb[band, kt] = t
        for band in range(3):
            nc.sync.dma_start(at_sb[band, 0][:], a_t[band, 0])

        def _vcopy(out, in_):
            nc.vector.tensor_copy(out=out, in_=in_)

        def _scopy(out, in_):
            nc.scalar.copy(out=out, in_=in_)

        # gpsimd (Pool) cannot read PSUM, so only DVE + Act move psum data
        copy_fns = [_vcopy, _scopy]
        eng_i = 0

        def next_engine():
            nonlocal eng_i
            e = copy_fns[eng_i % 2]
            eng_i += 1
            return e

        def load_chunk(rb, b, ht, wc):
            """One SBUF tile [128 h, 128 w, 36 c] fp16 per (ht, wc)
            quarter, so the first matmul only waits on one chunk."""
            t = xpool.tile([128, 128 * C], FP16,
                           name=f"x_{rb}_{ht}_{wc}", tag=f"x_{ht}_{wc}")
            dst = t.rearrange("h (w c) -> h w c", c=C)
            src = x[b, ht * 128:(ht + 1) * 128,
                    wc * 128:(wc + 1) * 128, :]
            nc.sync.dma_start(dst, src)
            return t

        # first chunk of image 0, then the kt=1 operator tiles, then the
        # rest -- keeps the PE's critical path fed as early as possible
        xt = {}
        xt[0, 0, 0] = load_chunk(0, 0, 0, 0)
        for band in range(3):
            nc.sync.dma_start(at_sb[band, 1][:], a_t[band, 1])
        for ht, wc in ((1, 0), (0, 1), (1, 1)):
            xt[0, ht, wc] = load_chunk(0, 0, ht, wc)

        # PE warm-up: ~10 dummy matmuls on the already-resident operator
        # tile while the first x chunks stream in, so the HAM clock gate
        # reaches 2.4 GHz before real work starts (window is ~3.4 us).
        warm_ps = psY.tile([128, 512], F32, name="warm_ps", tag="psY")
        for i in range(10):
            nc.tensor.matmul(warm_ps[:], at_sb[0, 0][:, 0:128],
                             at_sb[0, 0][:], start=True, stop=True)

        for rep in range(repeat):
          for b in range(BPC):
            rb = rep * BPC + b
            if rb + 1 < repeat * BPC:
                nb = (rb + 1) % BPC
                for ht, wc in ((0, 0), (1, 0), (0, 1), (1, 1)):
                    xt[rb + 1, ht, wc] = load_chunk(rb + 1, nb, ht, wc)

            xv = {(ht, wc): xt[rb, ht, wc].rearrange("h (w c) -> h w c",
                                                     c=C)
                  for ht in range(2) for wc in range(2)}

            # ---- stage Y: contract h -> u[q][wc] = [128 w, 512 h2] ----
            u_sb = {}
            for wc in range(2):
                for g2 in range(4):
                    for bx in range(3):
                        q = 3 * g2 + bx
                        P = psY.tile([128, 512], F32,
                                     name=f"psY_{rb}_{wc}_{q}", tag="psY")
                        for i, (by, ht) in enumerate(MM_ORDER):
                            c0q, c1q = KT_COLS[ht]
                            c0 = 9 * g2 + 3 * by + bx
                            lhsT = xv[ht, wc][:, :, c0]
                            nc.tensor.matmul(
                                P[:, c0q:c1q], lhsT,
                                at_sb[by, ht][:, c0q:c1q],
                                start=(i == 0), stop=(i == 5))
                        ut = upool.tile([128, 512], FP16,
                                        name=f"u_{rb}_{q}_{wc}",
                                        tag=f"u_{q}_{wc}")
                        next_engine()(ut[:], P[:])
                        u_sb[q, wc] = ut

            # ---- stage X: contract w -> psum [128 h2, 512 w2] ----
            for hb in range(4):
                osb = opool.tile([128, W2 * 4], FP16,
                                 name=f"osb_{rb}_{hb}", tag=f"osb_{hb}")
                osbv = osb.rearrange("p (w g) -> p g w", g=4)
                for g2 in range(4):
                    V = psX.tile([128, 512], F32,
                                 name=f"psX_{rb}_{hb}_{g2}", tag="psX")
                    for i, (bx, wc) in enumerate(MM_ORDER):
                        c0q, c1q = KT_COLS[wc]
                        lhsT = u_sb[3 * g2 + bx, wc][
                            :, hb * 128:(hb + 1) * 128]
                        nc.tensor.matmul(
                            V[:, c0q:c1q], lhsT,
                            at_sb[bx, wc][:, c0q:c1q],
                            start=(i == 0), stop=(i == 5))
                    next_engine()(osbv[:, g2, :], V[:])
                dstd = out[b, hb * 128:(hb + 1) * 128, :, :].rearrange(
                    "h w c -> h (w c)")
                nc.sync.dma_start(dstd, osb[:])
    nc.compile()
    return nc


_PROGRAMS = {}


def _get_program(repeat=1):
    if repeat not in _PROGRAMS:
        _PROGRAMS[repeat] = _build_program(repeat)
    return _PROGRAMS[repeat]


def _host_inputs(inputs):
    at = _build_operator_tiles()
    shards = inputs.astype(np.float16).reshape(
        NCORES, BPC, H, W, C)
    return [{"x": np.ascontiguousarray(shards[c]), "a_t": at}
            for c in range(NCORES)]


def _run(inputs, trace=False, tmpdir=None, repeat=1):
    """Returns (full output [16,512,512,4], BassKernelResults)."""
    inputs = np.ascontiguousarray(np.asarray(inputs, dtype=np.float32))
    assert inputs.shape == (B, H, W, C), inputs.shape
    nc = _get_program(repeat)
    in_maps = _host_inputs(inputs)
    res = run_bass_kernel_spmd(nc, in_maps, core_ids=list(range(NCORES)),
                               trace=trace, tmpdir=tmpdir)
    outs = [np.asarray(res.results[c]["out"]) for c in range(NCORES)]
    full = np.concatenate(outs, axis=0).astype(np.float32)
    return full, res


def kernel(inputs):
    full, _ = _run(inputs)
    return full


# revision 18
# speedup vs baseline: 3.9387x; 1.0033x over previous
"""Inverse separable wavelet synthesis (stride-2 transposed conv, 9 taps,
36 -> 12 -> 4 channels, 256x256 -> 512x512) on 8 trn2 NeuronCores.

Formulation: both passes are matmuls against the SAME host-precomputed
banded operator matrix A_band [256 in, 512 out] (symmetric padding +
border-mask sign folded in; H == W so both passes share the operators).

Data-stationary layout (the key trick): the *image data* is the
stationary (lhsT) operand and the *operator* is the moving (rhs)
operand.  The PE computes out[m,n] = lhsT.T @ rhs, so the psum
partition dim m comes from the data tile's free axis:

  stage Y: lhsT = x[h-rows, w-cols] (per channel), rhs = A_by[h, h2]
           -> psum [w, h2]   : u arrives ALREADY transposed for stage X
  stage X: lhsT = u[w-rows, h2-cols], rhs = A_bx[w, w2]
           -> psum [h2, w2]  : final output orientation, no transposes

Banded structure is exploited on the free axis: the k-split over two
128-row h (or w) tiles only needs output columns [0,260) / [252,512)
(5-tap window), so accumulation passes stream ~260 columns, not 512.
The first matmul of each group streams the full 512 columns so every
psum element is initialized by a start=True write (rhs is zero outside
the band, so this is exact).

All matmul operands are fp16: the operator coefficients are dyadic
rationals (exact in fp16); only x and the intermediate u are rounded
(~1e-3 relative output error).  Accumulation is exact fp32 in PSUM.
Input is converted to fp16 on the host, halving input DMA bytes; all
DMA tiles have 128 partitions so transfers spray across all 16 DMA
engines (a 68-partition tile would use only 4 = gcd-style spray rule).
"""

import numpy as np
from contextlib import ExitStack

import concourse.bass as bass
import concourse.bacc as bacc
import concourse.mybir as mybir
import concourse.tile as tile
from concourse.bass_utils import run_bass_kernel_spmd

B, H, W, C = 16, 256, 256, 36
NCORES = 8
BPC = B // NCORES  # images per core
W2 = 2 * W
H2 = 2 * H
F32 = mybir.dt.float32
FP16 = mybir.dt.float16

SMOOTH = [0.0, 0.0, 1.0 / 16.0, 0.5, 14.0 / 16.0, 0.5, 1.0 / 16.0, 0.0, 0.0]
EVEN = [-1.0 / 128.0, -1.0 / 16.0, -10.0 / 64.0, -7.0 / 16.0, 85.0 / 64.0,
        -7.0 / 16.0, -10.0 / 64.0, -1.0 / 16.0, -1.0 / 128.0]
ODD = [1.0 / 256.0, 1.0 / 32.0, 15.0 / 128.0, 17.0 / 32.0, 0.0,
       -17.0 / 32.0, -15.0 / 128.0, -1.0 / 32.0, -1.0 / 256.0]

# Output-column ranges fed by each 128-row k-tile: A[i, m] nonzero iff
# m in [2i-4, 2i+4], so k-tile 0 (rows 0..127) covers m < 259 and
# k-tile 1 (rows 128..255) covers m >= 252.
KT_COLS = {0: (0, 260), 1: (252, 512)}


def _build_operator_full():
    """[3 bands, 256 in-rows, 512 out-cols] float64 folded operator."""
    inv = np.array([SMOOTH, EVEN, ODD], dtype=np.float64)
    S = 256
    Sp = S + 6
    j = np.arange(Sp)[:, None]
    m = np.arange(2 * S)[None, :]
    t = m + 10 - 2 * j
    valid = (t >= 0) & (t <= 8)
    P = np.zeros((3, Sp, 2 * S))
    for b in range(3):
        P[b][valid] = inv[b][t[valid]]
    # border mask: odd band negated on the 3-wide padded border
    P[2, [0, 1, 2, Sp - 3, Sp - 2, Sp - 1], :] *= -1.0
    # fold symmetric padding: pad[0..2]=x[2],x[1],x[0]; pad[-3:]=x[-1],x[-2],x[-3]
    A = P[:, 3:3 + S].copy()
    A[:, 2] += P[:, 0]
    A[:, 1] += P[:, 1]
    A[:, 0] += P[:, 2]
    A[:, S - 1] += P[:, Sp - 3]
    A[:, S - 2] += P[:, Sp - 2]
    A[:, S - 3] += P[:, Sp - 1]
    return A


def _build_operator_tiles():
    """Partition-major packed operator [128, 3 bands * 2 ktiles * 512]
    fp16 -- 6144B contiguous per DRAM row, so the whole operator loads
    as ONE efficient DMA (the [.., 128, 512] layout would move in slow
    1KB packets)."""
    A = _build_operator_full()
    At = A.reshape(3, 2, 128, 512).transpose(2, 0, 1, 3).reshape(128, -1)
    return np.ascontiguousarray(At.astype(np.float16))


def _build_program(repeat=1):
    nc = bacc.Bacc("TRN2", target_bir_lowering=False)
    x = nc.declare_dram_parameter("x", [BPC, H, W, C], FP16, isOutput=False)
    a_t = nc.declare_dram_parameter("a_t", [128, 3 * 2 * 512], FP16,
                                    isOutput=False)
    out = nc.declare_dram_parameter("out", [BPC, H2, W2, 4], FP16,
                                    isOutput=True)

    # accumulation order for one psum tile: (band, ktile)
    MM_ORDER = [(0, 0), (1, 0), (2, 0), (0, 1), (1, 1), (2, 1)]

    with tile.TileContext(nc) as tc, ExitStack() as ctx:
        const = ctx.enter_context(tc.tile_pool(name="const", bufs=1))
        xpool = ctx.enter_context(tc.tile_pool(name="xp", bufs=2))
        upool = ctx.enter_context(tc.tile_pool(name="up", bufs=2))
        opool = ctx.enter_context(tc.tile_pool(name="op", bufs=1))
        psY = ctx.enter_context(tc.tile_pool(name="psY", bufs=4,
                                             space="PSUM"))
        psX = ctx.enter_context(tc.tile_pool(name="psX", bufs=4,
                                             space="PSUM"))

        # whole packed operator in one DMA; views per (band, ktile)
        at_all = const.tile([128, 3 * 2 * 512], FP16, name="at_all",
                            tag="at_all")
        nc.sync.dma_start(at_all[:], a_t[:])
        at_v = at_all.rearrange("p (b k n) -> p b k n", b=3, k=2)
        at_sb = {(band, kt): at_v[:, band, kt, :]
                 for band in range(3) for kt in range(2)}

        # PE warm-up fodder (memset on the otherwise-idle gpsimd engine,
        # so the dummy matmuls have no DMA dependency)
        warm_in = const.tile([128, 512], FP16, name="warm_in",
                             tag="warm_in")
        nc.gpsimd.memset(warm_in[:], 0.0)

        def _vcopy(out, in_):
            nc.vector.tensor_copy(out=out, in_=in_)

        def _scopy(out, in_):
            nc.scalar.copy(out=out, in_=in_)

        # gpsimd (Pool) cannot read PSUM, so only DVE + Act move psum data
        copy_fns = [_vcopy, _scopy]
        eng_i = 0

        def next_engine():
            nonlocal eng_i
            e = copy_fns[eng_i % 2]
            eng_i += 1
            return e

        def load_chunk(rb, b, ht, wc):
            """One SBUF tile [128 h, 128 w, 36 c] fp16 per (ht, wc)
            quarter, so the first matmul only waits on one chunk."""
            t = xpool.tile([128, 128 * C], FP16,
                           name=f"x_{rb}_{ht}_{wc}", tag=f"x_{ht}_{wc}")
            dst = t.rearrange("h (w c) -> h w c", c=C)
            src = x[b, ht * 128:(ht + 1) * 128,
                    wc * 128:(wc + 1) * 128, :]
            nc.sync.dma_start(dst, src)
            return t

        xt = {}
        for ht, wc in ((0, 0), (1, 0), (0, 1), (1, 1)):
            xt[0, ht, wc] = load_chunk(0, 0, ht, wc)

        # PE warm-up: dummy matmuls with no data dependency (uninit
        # warm_in) run during the initial DMA wait, so the HAM clock
        # gate reaches 2.4 GHz before real work starts (~3.4 us window).
        warm_ps = psY.tile([128, 512], F32, name="warm_ps", tag="psY")
        for i in range(10):
            nc.tensor.matmul(warm_ps[:], warm_in[:, 0:128],
                             warm_in[:], start=True, stop=True)

        for rep in range(repeat):
          for b in range(BPC):
            rb = rep * BPC + b
            if rb + 1 < repeat * BPC:
                nb = (rb + 1) % BPC
                for ht, wc in ((0, 0), (1, 0), (0, 1), (1, 1)):
                    xt[rb + 1, ht, wc] = load_chunk(rb + 1, nb, ht, wc)

            xv = {(ht, wc): xt[rb, ht, wc].rearrange("h (w c) -> h w c",
                                                     c=C)
                  for ht in range(2) for wc in range(2)}

            # ---- stage Y: contract h -> u[q][wc] = [128 w, 512 h2] ----
            u_sb = {}
            for wc in range(2):
                for g2 in range(4):
                    for bx in range(3):
                        q = 3 * g2 + bx
                        P = psY.tile([128, 512], F32,
                                     name=f"psY_{rb}_{wc}_{q}", tag="psY")
                        for i, (by, ht) in enumerate(MM_ORDER):
                            c0q, c1q = KT_COLS[ht]
                            c0 = 9 * g2 + 3 * by + bx
                            lhsT = xv[ht, wc][:, :, c0]
                            nc.tensor.matmul(
                                P[:, c0q:c1q], lhsT,
                                at_sb[by, ht][:, c0q:c1q],
                                start=(i == 0), stop=(i == 5))
                        ut = upool.tile([128, 512], FP16,
                                        name=f"u_{rb}_{q}_{wc}",
                                        tag=f"u_{q}_{wc}")
                        next_engine()(ut[:], P[:])
                        u_sb[q, wc] = ut

            # ---- stage X: contract w -> psum [128 h2, 512 w2] ----
            for hb in range(4):
                osb = opool.tile([128, W2 * 4], FP16,
                                 name=f"osb_{rb}_{hb}", tag=f"osb_{hb}")
                osbv = osb.rearrange("p (w g) -> p g w", g=4)
                # final tile of the kernel: split copies + DMA by
                # partition halves to shorten the drain tail
                last = (rb == repeat * BPC - 1 and hb == 3)
                for g2 in range(4):
                    V = psX.tile([128, 512], F32,
                                 name=f"psX_{rb}_{hb}_{g2}", tag="psX")
                    for i, (bx, wc) in enumerate(MM_ORDER):
                        c0q, c1q = KT_COLS[wc]
                        lhsT = u_sb[3 * g2 + bx, wc][
                            :, hb * 128:(hb + 1) * 128]
                        nc.tensor.matmul(
                            V[:, c0q:c1q], lhsT,
                            at_sb[bx, wc][:, c0q:c1q],
                            start=(i == 0), stop=(i == 5))
                    if last:
                        nc.vector.tensor_copy(out=osbv[0:64, g2, :],
                                              in_=V[0:64, :])
                        nc.scalar.copy(out=osbv[64:128, g2, :],
                                       in_=V[64:128, :])
                    else:
                        next_engine()(osbv[:, g2, :], V[:])
                dstd = out[b, hb * 128:(hb + 1) * 128, :, :].rearrange(
                    "h w c -> h (w c)")
                if last:
                    nc.sync.dma_start(dstd[0:64, :], osb[0:64, :])
                    nc.sync.dma_start(dstd[64:128, :], osb[64:128, :])
                else:
                    nc.sync.dma_start(dstd, osb[:])
    nc.compile()
    return nc


_PROGRAMS = {}


def _get_program(repeat=1):
    if repeat not in _PROGRAMS:
        _PROGRAMS[repeat] = _build_program(repeat)
    return _PROGRAMS[repeat]


def _host_inputs(inputs):
    at = _build_operator_tiles()
    shards = inputs.astype(np.float16).reshape(
        NCORES, BPC, H, W, C)
    return [{"x": np.ascontiguousarray(shards[c]), "a_t": at}
            for c in range(NCORES)]


def _run(inputs, trace=False, tmpdir=None, repeat=1):
    """Returns (full output [16,512,512,4], BassKernelResults)."""
    inputs = np.ascontiguousarray(np.asarray(inputs, dtype=np.float32))
    assert inputs.shape == (B, H, W, C), inputs.shape
    nc = _get_program(repeat)
    in_maps = _host_inputs(inputs)
    res = run_bass_kernel_spmd(nc, in_maps, core_ids=list(range(NCORES)),
                               trace=trace, tmpdir=tmpdir)
    outs = [np.asarray(res.results[c]["out"]) for c in range(NCORES)]
    full = np.concatenate(outs, axis=0).astype(np.float32)
    return full, res


def kernel(inputs):
    full, _ = _run(inputs)
    return full
